# revision 7
# baseline (speedup 1.0000x reference)
"""GRU (Flax GRUCell scanned over time) on 8 Trainium2 NeuronCores.

Problem: x:[T,B,D]=[512,64,512], h0:[B,H], Wi:[D,3H], Wh:[H,3H], bi:[3H], bhn:[H]
  gi = x_t @ Wi + bi ; gh = h @ Wh ; gates (r,z,n); h' = (1-z)*n + z*h
  returns ys:[T,B,H] (the h trajectory).

v2 strategy (per core, data-parallel over batch, B_local=8):
  T-layout on chip: hidden dim on SBUF partitions, batch on the free dim.

  1. gi PRE-PASS: gi[t] = Wi.T @ x_t for ALL t up front, with 512-wide
     moving operands (full PE efficiency, one weight load per (m,k,block)
     instead of one per step).  Results stored bf16 in SBUF.
  2. Per step, only the Wh matmuls remain on the PE (48 LDWEIGHTS+MATMUL
     pairs), plus ONE identity-stationary matmul that injects gi's r/z
     rows into the gate PSUM (free add via PSUM accumulation).
  3. GN-way interleaved recurrences over disjoint batch groups: while the
     PE runs group g's Wh block, the Act/DVE/Pool engines run the other
     groups' gate chains.  The serial gate chain hides behind PE work.

  Gate chain per (group, step):
    sigmoid(rz) [Act, PSUM->PSUM] -> rpn = gh_n * r [DVE] ->
    pre_n = rpn + gi_n(bf16 SBUF) [DVE] -> tanh [Act] ->
    w/hb in 2 half-chunks [DVE];  off-chain on Pool: omz = 1-z,
    v = z*h_prev, h' = w+v (fp32, to the output ring).
"""

import warnings

warnings.filterwarnings("ignore")

import numpy as np
import ml_dtypes

import concourse.bacc as bacc
import concourse.tile as tile
from concourse import mybir, bass_utils

B, D, H = 64, 512, 512
NCORES = 8
BL = B // NCORES  # batch per core
KD = D // 128  # input-dim k-chunks
KH = H // 128  # hidden-dim k-chunks
M3 = 3 * H // 128  # 3H m-tiles
RT = 8  # output-ring steps per DMA
GN = 3  # interleaved batch groups per core
TB_PRE = 512  # prepass moving width (columns)
BF16 = mybir.dt.bfloat16
F32 = mybir.dt.float32
NPBF16 = ml_dtypes.bfloat16

_cache: dict = {}


def _gslices(gn):
    # split BL into gn contiguous batch groups, sizes as even as possible
    base = BL // gn
    rem = BL % gn
    out = []
    o = 0
    for g in range(gn):
        sz = base + (1 if g < rem else 0)
        out.append((o, o + sz))
        o += sz
    return out


def _build(T: int, use_bi: bool, use_bhn: bool, gn: int = GN):
    TB = T * BL
    assert T % RT == 0
    tb_pre = min(TB_PRE, TB)
    nblk = TB // tb_pre
    assert TB % tb_pre == 0
    tpb = tb_pre // BL  # steps per prepass block
    nc = bacc.Bacc("TRN2", target_bir_lowering=False, debug=False, num_devices=NCORES)

    xt_d = nc.dram_tensor("xt", [128, KD * TB], BF16, kind="ExternalInput").ap()
    wi_d = nc.dram_tensor("wi", [128, M3 * KD * 128], BF16, kind="ExternalInput").ap()
    wh_d = nc.dram_tensor("wh", [128, M3 * KH * 128], BF16, kind="ExternalInput").ap()
    h0_d = nc.dram_tensor("h0t", [128, KH * BL], F32, kind="ExternalInput").ap()
    eye_d = nc.dram_tensor("eye", [128, 128], BF16, kind="ExternalInput").ap()
    bi_d = (
        nc.dram_tensor("bi_r", [1, M3 * 128], BF16, kind="ExternalInput").ap()
        if use_bi
        else None
    )
    bhn_d = (
        nc.dram_tensor("bhn_t", [128, KH], F32, kind="ExternalInput").ap()
        if use_bhn
        else None
    )
    ys_d = nc.dram_tensor("yst", [128, KH * TB], F32, kind="ExternalOutput").ap()
    ys_v = ys_d.rearrange("p (k t j) -> p k t j", k=KH, j=BL)

    gsl = _gslices(gn)

    with tile.TileContext(nc) as tc:
        with (
            tc.tile_pool(name="const", bufs=1) as const,
            tc.tile_pool(name="xin", bufs=1) as xin,
            tc.tile_pool(name="gip", bufs=1) as gip,
            tc.tile_pool(name="orp", bufs=3) as orp,
            tc.tile_pool(name="ew", bufs=2) as ew,
        ):
            # ---- load constants ----
            wi_sb = const.tile([128, M3 * KD * 128], BF16)
            nc.sync.dma_start(wi_sb[:], wi_d[:])
            wh_sb = const.tile([128, M3 * KH * 128], BF16)
            nc.sync.dma_start(wh_sb[:], wh_d[:])
            eye_sb = const.tile([128, 128], BF16)
            nc.sync.dma_start(eye_sb[:], eye_d[:])
            h0_sb = const.tile([128, KH, BL], F32)
            nc.sync.dma_start(h0_sb[:], h0_d.rearrange("p (k j) -> p k j", j=BL))
            if use_bi:
                bi_sb = const.tile([1, M3 * 128], BF16)
                nc.sync.dma_start(bi_sb[:], bi_d[:])
                ones_sb = const.tile([1, tb_pre], BF16)
                nc.vector.memset(ones_sb[:], 1.0)
            if use_bhn:
                bhn_sb = const.tile([128, KH], F32)
                nc.sync.dma_start(bhn_sb[:], bhn_d[:])
            xt_sb = xin.tile([128, KD * TB], BF16)
            nc.sync.dma_start(xt_sb[:], xt_d[:])

            # gi for all steps, bf16, [128, T, M3, BL]
            gi_sb = gip.tile([128, T, M3, BL], BF16)

            # ---- gi prepass ----
            with tc.tile_pool(name="pp", bufs=2, space="PSUM") as ppp:
                ci = 0
                for m in range(M3):
                    for tb in range(nblk):
                        pp = ppp.tile([128, tpb, BL], F32, tag="pp")
                        for k in range(KD):
                            nc.tensor.matmul(
                                pp[:],
                                wi_sb[:, (m * KD + k) * 128 : (m * KD + k + 1) * 128],
                                xt_sb[:, k * TB + tb * tb_pre : k * TB + (tb + 1) * tb_pre],
                                start=(k == 0),
                                stop=(k == KD - 1) and not use_bi,
                                skip_group_check=True,
                            )
                        if use_bi:
                            nc.tensor.matmul(
                                pp[:],
                                bi_sb[:, m * 128 : (m + 1) * 128],
                                ones_sb[:],
                                start=False,
                                stop=True,
                                skip_group_check=True,
                            )
                        dst = gi_sb[:, tb * tpb : (tb + 1) * tpb, m, :]
                        if ci % 2 == 0:
                            nc.scalar.activation(
                                dst, pp[:], mybir.ActivationFunctionType.Copy
                            )
                        else:
                            nc.vector.tensor_copy(dst, pp[:])
                        ci += 1

            # ---- recurrence ----
            with (
                tc.tile_pool(name="gpp", bufs=1, space="PSUM") as gpp,
                tc.tile_pool(name="hbp", bufs=2) as hbp,
            ):
                hb = []
                h_prev = []
                for g, (b0, b1) in enumerate(gsl):
                    hbg = hbp.tile([128, KH, b1 - b0], BF16, tag=f"hb{g}")
                    nc.vector.tensor_copy(hbg[:], h0_sb[:, :, b0:b1])
                    hb.append(hbg)
                    h_prev.append(h0_sb[:, :, b0:b1])

                o_cur = None
                for t in range(T):
                    u = t % RT
                    if u == 0:
                        o_cur = orp.tile([128, KH, RT, BL], F32, tag="oring")

                    for g, (b0, b1) in enumerate(gsl):
                        gb = b1 - b0
                        # --- PE block: Wh matmuls + gi rz injection ---
                        gp = gpp.tile([128, M3, gb], F32, tag=f"gp{g}")
                        for k in range(KH):
                            ms = range(M3) if k < KH - 1 else range(8)
                            for m in ms:
                                nc.tensor.matmul(
                                    gp[:, m, :],
                                    wh_sb[:, (m * KH + k) * 128 : (m * KH + k + 1) * 128],
                                    hb[g][:, k, :],
                                    start=(k == 0 and m == 0),
                                    stop=False,
                                    skip_group_check=True,
                                )
                        # inject gi r/z rows (ends rz accumulation groups)
                        nc.tensor.matmul(
                            gp[:, 0:8, :],
                            eye_sb[:],
                            gi_sb[:, t, 0:8, b0:b1],
                            start=False,
                            stop=True,
                            skip_group_check=True,
                        )
                        for m in range(8, M3):
                            nc.tensor.matmul(
                                gp[:, m, :],
                                wh_sb[:, (m * KH + KH - 1) * 128 : (m * KH + KH) * 128],
                                hb[g][:, KH - 1, :],
                                start=False,
                                stop=True,
                                skip_group_check=True,
                            )

                        # --- gate chain ---
                        rzt = ew.tile([128, 8, gb], F32, tag=f"rzt{g}")
                        nc.scalar.activation(
                            rzt[:], gp[:, 0:8, :], mybir.ActivationFunctionType.Sigmoid
                        )
                        omz = ew.tile([128, KH, gb], F32, tag=f"omz{g}")
                        nc.gpsimd.tensor_scalar(
                            omz[:],
                            rzt[:, KH : 2 * KH, :],
                            -1.0,
                            1.0,
                            mybir.AluOpType.mult,
                            mybir.AluOpType.add,
                        )
                        v = ew.tile([128, KH, gb], F32, tag=f"v{g}")
                        nc.gpsimd.tensor_mul(v[:], rzt[:, KH : 2 * KH, :], h_prev[g])
                        rpn = ew.tile([128, KH, gb], BF16, tag=f"rpn{g}")
                        if use_bhn:
                            for kk in range(KH):
                                nc.vector.scalar_tensor_tensor(
                                    rpn[:, kk, :],
                                    gp[:, 8 + kk, :],
                                    bhn_sb[:, kk : kk + 1],
                                    rzt[:, kk, :],
                                    mybir.AluOpType.add,
                                    mybir.AluOpType.mult,
                                )
                        else:
                            nc.vector.tensor_mul(
                                rpn[:], gp[:, 8:12, :], rzt[:, 0:KH, :]
                            )
                        pre_n = ew.tile([128, KH, gb], F32, tag=f"pren{g}")
                        nc.vector.tensor_add(
                            pre_n[:], rpn[:], gi_sb[:, t, 8:12, b0:b1]
                        )
                        nt = ew.tile([128, KH, gb], F32, tag=f"nt{g}")
                        nc.scalar.activation(
                            nt[:], pre_n[:], mybir.ActivationFunctionType.Tanh
                        )
                        w = ew.tile([128, KH, gb], F32, tag=f"w{g}")
                        hbg = hbp.tile([128, KH, gb], BF16, tag=f"hb{g}")
                        for h2 in range(0, KH, 2):
                            nc.vector.tensor_mul(
                                w[:, h2 : h2 + 2, :],
                                nt[:, h2 : h2 + 2, :],
                                omz[:, h2 : h2 + 2, :],
                            )
                            nc.vector.tensor_add(
                                hbg[:, h2 : h2 + 2, :],
                                w[:, h2 : h2 + 2, :],
                                v[:, h2 : h2 + 2, :],
                            )
                        hb[g] = hbg
                        h_new = o_cur[:, :, u, b0:b1]
                        nc.gpsimd.tensor_add(h_new, w[:], v[:])
                        h_prev[g] = h_new

                    if u == RT - 1:
                        nc.sync.dma_start(
                            ys_v[:, :, t - RT + 1 : t + 1, :], o_cur[:]
                        )

    nc.compile()
    return nc


def _get(T, use_bi, use_bhn):
    key = (T, use_bi, use_bhn, GN)
    if key not in _cache:
        _cache[key] = _build(T, use_bi, use_bhn, GN)
    return _cache[key]


def _pack_w(W, kc):
    # W [kc*128, M3*128] -> [128, M3*kc*128], col ((m*kc)+k)*128+c = W[k*128+p, m*128+c]
    return np.ascontiguousarray(
        W.astype(NPBF16).reshape(kc, 128, M3, 128).transpose(1, 2, 0, 3).reshape(128, -1)
    )


def kernel(x, h0, Wi, Wh, bi, bhn, _trace=False, _trace_kwargs=None):
    T = x.shape[0]
    use_bi = bool(np.any(bi))
    use_bhn = bool(np.any(bhn))
    nc = _get(T, use_bi, use_bhn)
    TB = T * BL

    wi_p = _pack_w(np.asarray(Wi), KD)
    wh_p = _pack_w(np.asarray(Wh), KH)
    eye = np.ascontiguousarray(np.eye(128, dtype=NPBF16))
    x = np.asarray(x)
    h0 = np.asarray(h0)

    in_maps = []
    for c in range(NCORES):
        xc = x[:, c * BL : (c + 1) * BL, :]  # [T, BL, D]
        xt = np.ascontiguousarray(
            xc.astype(NPBF16).reshape(T, BL, KD, 128).transpose(3, 2, 0, 1).reshape(128, KD * TB)
        )
        h0c = np.ascontiguousarray(
            h0[c * BL : (c + 1) * BL, :].astype(np.float32).reshape(BL, KH, 128).transpose(2, 1, 0).reshape(128, KH * BL)
        )
        im = {"xt": xt, "wi": wi_p, "wh": wh_p, "h0t": h0c, "eye": eye}
        if use_bi:
            im["bi_r"] = np.ascontiguousarray(bi.astype(NPBF16).reshape(1, M3 * 128))
        if use_bhn:
            im["bhn_t"] = np.ascontiguousarray(bhn.astype(np.float32).reshape(KH, 128).T)
        in_maps.append(im)

    kw = {}
    if _trace:
        kw = dict(trace=True, **(_trace_kwargs or {}))
    kernel._last_in_maps = in_maps
    res = bass_utils.run_bass_kernel_spmd(nc, in_maps, core_ids=list(range(NCORES)), **kw)

    ys = np.empty((T, B, H), dtype=np.float32)
    for c in range(NCORES):
        out = res.results[c]["yst"]  # [128, KH*TB]
        ys[:, c * BL : (c + 1) * BL, :] = (
            out.reshape(128, KH, T, BL).transpose(2, 3, 1, 0).reshape(T, BL, H)
        )
    kernel._last_result = res
    return ys
